# revision 1
# baseline (speedup 1.0000x reference)
"""Distributed causal self-attention kernel for one TRN2 chip (8 NeuronCores).

Problem: B=2, T=2048, C=1024, H=16 heads, D=64. f32 in/out.

Sharding: DP=2 over batch x TP=4 over heads.
  core c -> (b = c//4, g = c%4), owns heads 4g..4g+3 of batch b.

Per-core device program (SPMD, identical graph on all 8 cores), built with
Tile and scheduled as one fused stream so the PE never idles long enough for
the HAM clock gate to re-throttle:

  - startup-critical loads (W_qk + the first x^T chunk) are spread over the
    Sync/Scalar/Vector/GpSimd sequencers: every dma_start costs ~0.65us of
    serial issue time on its engine, so 4-way splitting quarters the time to
    the first matmul.
  - qk^T = (x @ [W_q/8 | W_k])^T computed directly in transposed layout via
    matmul(lhsT=W_qk_tile, rhs=x^T_tile); x^T is fed pre-transposed from the
    host, so NO on-chip transposes are needed anywhere. float32r inputs
    (full-rate fp32 PE path), fp32 PSUM, bf16 evacuation fused with the
    per-partition q/k bias add.
  - v = x @ W_v in natural layout, head-interleaved with a ones column
    (memset once at startup) -> lhsT = [v_h | 1] so the attention AV matmul
    also produces the softmax row-sums for free (row 64 of the accumulator).
  - attention per 512-token chunk, heads in pairs: the even head's q/k rows
    sit at partitions 0-63 and the odd head's at 64-127, so interleaved S^T
    matmuls (K=64, bf16) alternate PE row groups and their weight loads
    overlap in-flight matmuls. exp on ScalarE ([128,1024] PSUM->SBUF, bf16
    out, softmax scale pre-folded into W_q on the host); causal masking of
    diagonal tiles via precomputed multiplicative bf16 masks on VectorE.
    No max-subtraction is needed: S = qk/sqrt(D) is O(5) here, exp is safe
    in fp32. Normalization: rowsum staged to SBUF, broadcast across
    partitions (GpSimd), fast reciprocal + multiply on VectorE, bf16 y^T.
  - projection WITHOUT gathering y: each core computes its PARTIAL
    out^T = W_p_local^T @ y_local per 512-token chunk (bf16, K=256), where
    W_p_local = W_proj rows for this core's heads, giving a [1024 out-ch,
    tok] partial that only needs summing across the 4 cores of the batch
    group. Partials are evacuated on GpSimd and stored rank-major
    ([1024 rows = 4 ranks x 256 ch]) in DRAM, then reduced with a 4-core
    ReduceScatter (groups {0-3}, {4-7}, bf16 add), so each core ends with
    its own [256 ch, tok] slice of the final output. vs. the all-gather
    design this cuts collective bytes 8x, halves projection FLOPs, and
    removes the gathered-y reload from DRAM entirely.
  - 2 ReduceScatter ships: chunks 0-2 ship once all three are projected
    (fully hidden behind chunk-3 attention), chunk 3 ships at the end (the
    only exposed collective, 0.25MB out). Ship points are chosen so nothing
    queued later on the GpSimd sequencer is needed before the collective
    completes. A tiny warmup collective at kernel start hides the ncfw
    cold-start. The RS results bounce DRAM->SBUF->output at the very end
    (collectives cannot write IO tensors directly).

Host: shards inputs (x transposed per batch, W_attn column-sliced with the
softmax scale folded into W_q, W_proj row-sliced to bf16), reassembles the
8 [256, 2048] bf16 out^T slices, adds b_proj (exact for the final linear
step).
"""

import numpy as np

import concourse.bass as bass
import concourse.bacc as bacc
import concourse.mybir as mybir
import concourse.tile as tile
from concourse import bass_utils

F32 = mybir.dt.float32
F32R = mybir.dt.float32r
BF16 = mybir.dt.bfloat16

B, T, C = 2, 2048, 1024
H, D = 16, 64
DP, TP = 2, 4
HPC = H // TP            # 4 heads per core
CH = HPC * D             # 256 channels per core
NCORES = DP * TP

RG4 = [[0, 1, 2, 3], [4, 5, 6, 7]]


def build_kernel(trace_sim: bool = False):
    nc = bacc.Bacc("TRN2", target_bir_lowering=False, debug=False,
                   num_devices=NCORES)

    x_t = nc.dram_tensor("x_t", [C, T], F32R, kind="ExternalInput").ap()
    w_qk = nc.dram_tensor("w_qk", [C, 2 * CH], F32R, kind="ExternalInput").ap()
    b_qk = nc.dram_tensor("b_qk", [2 * CH], F32, kind="ExternalInput").ap()
    w_v = nc.dram_tensor("w_v", [C, CH], F32R, kind="ExternalInput").ap()
    b_v = nc.dram_tensor("b_v", [CH], F32, kind="ExternalInput").ap()
    w_p = nc.dram_tensor("w_p", [CH, C], BF16, kind="ExternalInput").ap()
    # out^T slices per RS ship: [own 256 channels, ship tokens]
    rs_w = [1536, 512]
    outs = [nc.dram_tensor(f"o{s}", [CH, w], BF16, kind="ExternalOutput").ap()
            for s, w in enumerate(rs_w)]

    KT = C // 128        # 8 contraction tiles for C
    NTT = T // 128       # 16 token tiles
    NTC = T // 512       # 4 token chunks

    from contextlib import ExitStack
    with tile.TileContext(nc, trace_sim=trace_sim) as tc, ExitStack() as ctx:
        const = ctx.enter_context(tc.tile_pool(name="const", bufs=1))
        qkp = ctx.enter_context(tc.tile_pool(name="qkp", bufs=1))
        vp = ctx.enter_context(tc.tile_pool(name="vp", bufs=1))
        yp = ctx.enter_context(tc.tile_pool(name="yp", bufs=1))
        ep = ctx.enter_context(tc.tile_pool(name="ep", bufs=4))
        rbp = ctx.enter_context(tc.tile_pool(name="rbp", bufs=2))
        wpp = ctx.enter_context(tc.tile_pool(name="wpp", bufs=1))
        osb = ctx.enter_context(tc.tile_pool(name="osb", bufs=3))
        dram = ctx.enter_context(tc.tile_pool(name="dram", bufs=1, space="DRAM"))
        xp = ctx.enter_context(tc.tile_pool(name="xp", bufs=1))

        # ---- persistent SBUF tensors -------------------------------------
        Wqk = [const.tile([128, 2 * CH], F32R, name=f"wqk{k}") for k in range(KT)]
        Wv = [const.tile([128, CH], F32R, name=f"wv{k}") for k in range(KT)]
        Wp = [wpp.tile([128, C], BF16, name=f"wp{k}") for k in range(2)]
        bqk = const.tile([128, 4], F32, name="bqk")
        bvrow = const.tile([1, CH], F32, name="bvrow")
        bvbc = const.tile([128, CH], F32, name="bvbc")
        dmask = [const.tile([128, 1024], BF16, name=f"dmask{r}") for r in range(2)]
        xT = [[xp.tile([128, 512], F32R, name=f"xT{k}_{j}") for j in range(NTC)]
              for k in range(KT)]
        # qk^T tiles (bf16): [o-tile i][t-chunk j] -> [128, 512]
        # i = 0,1: q rows (pre-scaled by 1/sqrt(D) on host); i = 2,3: k rows
        qkT = [[qkp.tile([128, 512], BF16, name=f"qkT{i}_{j}") for j in range(NTC)]
               for i in range(4)]
        # v tiles (bf16), head-interleaved with a ones column: [128, 4*65]
        v_sb = [vp.tile([128, HPC * (D + 1)], BF16, name=f"v{m}") for m in range(NTT)]
        # normalized y^T chunk tiles (bf16): [chunk j] -> [256, 512] as 2x128
        yT = [[yp.tile([128, 512], BF16, name=f"yT{i}_{j}") for j in range(NTC)]
              for i in range(2)]

        cc_win = dram.tile([8, 16], BF16, name="cc_win")
        cc_wout = dram.tile([2, 16], BF16, name="cc_wout")
        warm_sb = const.tile([8, 16], BF16, name="warm_sb")
        act_warm = const.tile([1, 16], F32, name="act_warm")
        # ReduceScatter buffers, rank-major rows = 1024 out channels:
        # ship 0 = chunks 0-2 (1536 tok), ship 1 = chunk 3 (512 tok). The RS
        # result lands in a Local DRAM bounce (the verifier forbids
        # collectives writing IO tensors) and goes via SBUF to the output.
        rs_in = [dram.tile([TP * CH, w], BF16, name=f"rs_in{s}")
                 for s, w in enumerate(rs_w)]
        rs_out = [dram.tile([CH, w], BF16, name=f"rs_out{s}")
                  for s, w in enumerate(rs_w)]

        # ---- startup: spread the critical loads over 4 sequencers --------
        nc.sync.dma_start(bvrow[:], b_v.unsqueeze(0))
        load_eng = [nc.sync, nc.sync, nc.sync, nc.sync,
                    nc.scalar, nc.scalar, nc.scalar, nc.scalar]
        for k in range(KT):
            load_eng[k].dma_start(Wqk[k][:], w_qk[128 * k:128 * k + 128, :])
            load_eng[k].dma_start(xT[k][0][:], x_t[128 * k:128 * k + 128, 0:512])
        nc.sync.dma_start(bqk[:], b_qk.rearrange("(i p) -> p i", p=128))

        # GpSimd: bias broadcast + causal masks, then the warmup collective
        # (pays the ~11us ncfw cold-start while the QKV phase runs). Nothing
        # later on GpSimd is needed before it completes.
        nc.gpsimd.partition_broadcast(bvbc[:], bvrow[:])
        for r in range(2):
            nc.gpsimd.memset(dmask[r][:], 1.0)
            nc.gpsimd.affine_select(
                out=dmask[r][:], in_=dmask[r][:],
                compare_op=mybir.AluOpType.is_ge, fill=0.0,
                base=-256 * r, pattern=[[-128, 2], [1, 512]],
                channel_multiplier=-1)
        nc.vector.memset(warm_sb[:], 0.0)
        nc.sync.dma_start(cc_win[:], warm_sb[:])
        nc.gpsimd.collective_compute(
            "ReduceScatter", mybir.AluOpType.add, replica_groups=RG4,
            ins=[cc_win.opt()], outs=[cc_wout.opt()])

        # Vector: dummy exp operand (pulls the ~2.7us ACT table load into the
        # DMA phase), the ones columns of the v tiles, and the ones row for
        # the PE-broadcast normalize of chunk 3
        nc.vector.memset(act_warm[:], 0.0)
        nc.scalar.activation(act_warm[:], act_warm[:],
                             mybir.ActivationFunctionType.Exp)
        ones64 = const.tile([1, D], BF16, name="ones64")
        nc.vector.memset(ones64[:], 1.0)
        ones4 = const.tile([128, 4], BF16, name="ones4")
        nc.vector.memset(ones4[:], 1.0)
        for m in range(NTT):
            ones_ap = v_sb[m].rearrange("p (h x) -> p h x", x=D + 1)[:, :, D:D + 1]
            nc.vector.tensor_copy(
                ones_ap, ones4.rearrange("p (h x) -> p h x", x=1))

        # bulk loads: remaining x^T chunks, W_v, W_proj — all on Sync, which
        # is otherwise idle (Scalar must stay clear for the chunk-0 exps)
        for k in range(KT):
            nc.sync.dma_start(Wv[k][:], w_v[128 * k:128 * k + 128, :])
        for k in range(2):
            nc.sync.dma_start(Wp[k][:], w_p[128 * k:128 * k + 128, :])
        for j in range(1, NTC):
            for k in range(KT):
                nc.sync.dma_start(xT[k][j][:],
                                  x_t[128 * k:128 * k + 128,
                                      512 * j:512 * j + 512])

        # ---- per-chunk phases --------------------------------------------
        def qk_group(j):
            # qk^T = W_qk^T @ x^T for one token chunk
            for i in range(4):
                ps = psM.tile([128, 512], F32, name="psA", tag="psM")
                for k in range(KT):
                    nc.tensor.matmul(
                        ps[:],
                        Wqk[k][:, 128 * i:128 * i + 128],
                        xT[k][j][:],
                        start=(k == 0), stop=(k == KT - 1))
                nc.vector.tensor_scalar_add(qkT[i][j][:], ps[:], bqk[:, i:i + 1])

        def v_group(j):
            # v = x @ W_v (natural layout, +bias, head-interleaved ones col)
            for m in range(4 * j, 4 * j + 4):
                ps = psM.tile([128, CH], F32, name="psB", tag="psM")
                for k in range(KT):
                    nc.tensor.matmul(
                        ps[:],
                        xT[k][m // 4][:, 128 * (m % 4):128 * (m % 4) + 128],
                        Wv[k][:],
                        start=(k == 0), stop=(k == KT - 1))
                v_ap = v_sb[m].rearrange("p (h x) -> p h x", x=D + 1)[:, :, 0:D]
                nc.vector.tensor_add(
                    v_ap,
                    ps.rearrange("p (h d) -> p h d", d=D),
                    bvbc.rearrange("p (h d) -> p h d", d=D))

        # Heads are processed in pairs (2hp, 2hp+1). The even head's q/k rows
        # live at partitions 0-63, the odd head's at 64-127, so interleaved S
        # matmuls alternate PE row groups (tile_position auto-derives from
        # base_partition): the next weight load overlaps the in-flight matmul
        # and the two K=64 matmuls stream concurrently.
        def attn_chunk(j):
            for hp in range(HPC // 2):
                attn_pair(j, hp)

        def attn_pair(j, hp):
                ha, hb = 2 * hp, 2 * hp + 1
                y_psA = psY.tile([D + 1, 512], F32, name="y_psA", tag="y_ps")
                y_psB = psY.tile([D + 1, 512], F32, name="y_psB", tag="y_ps")
                n_s = 4 * (j + 1)           # causal s-tiles for this chunk
                for sp in range(n_s // 2):  # pairs of 128-row s-tiles
                    sA = psS.tile([128, 1024], F32, name="sA", tag="s_ps")
                    sB = psS.tile([128, 1024], F32, name="sB", tag="s_ps")
                    eA = ep.tile([128, 1024], BF16, name="eA", tag="e_sb")
                    eB = ep.tile([128, 1024], BF16, name="eB", tag="e_sb")
                    for half in range(2):
                        st = 2 * sp + half
                        kt = qkT[2 + hp][st // 4]
                        qt = qkT[hp][j]
                        ks = 128 * (st % 4)
                        nc.tensor.matmul(
                            sA[:, 512 * half:512 * half + 512],
                            kt[0:64, ks:ks + 128], qt[0:64, :],
                            start=True, stop=True)
                        nc.tensor.matmul(
                            sB[:, 512 * half:512 * half + 512],
                            kt[64:128, ks:ks + 128], qt[64:128, :],
                            start=True, stop=True)
                    nc.scalar.activation(
                        eA[:], sA[:], mybir.ActivationFunctionType.Exp)
                    nc.scalar.activation(
                        eB[:], sB[:], mybir.ActivationFunctionType.Exp)
                    if 2 * sp >= 4 * j:     # pair straddles the diagonal
                        r_idx = (2 * sp - 4 * j) // 2
                        for e in (eA, eB):
                            nc.vector.tensor_mul(e[:], e[:], dmask[r_idx][:])
                    for half in range(2):
                        st = 2 * sp + half
                        nc.tensor.matmul(
                            y_psA[:],
                            v_sb[st][:, (D + 1) * ha:(D + 1) * ha + D + 1],
                            eA[:, 512 * half:512 * half + 512],
                            start=(st == 0), stop=(st == n_s - 1))
                        nc.tensor.matmul(
                            y_psB[:],
                            v_sb[st][:, (D + 1) * hb:(D + 1) * hb + D + 1],
                            eB[:, 512 * half:512 * half + 512],
                            start=(st == 0), stop=(st == n_s - 1))
                # normalize: y * (1/rowsum); stage the rowsum in SBUF (GpSimd
                # can't read PSUM), broadcast across partitions on GpSimd,
                # fast reciprocal + multiply on VectorE
                for hh, y_ps in ((ha, y_psA), (hb, y_psB)):
                    y_dst = yT[hp][j][64 * (hh % 2):64 * (hh % 2) + 64, :]
                    r_sb = rbp.tile([1, 512], F32, name="r_sb", tag="r_sb")
                    nc.vector.tensor_copy(r_sb[:], y_ps[D:D + 1, :])
                    rbc = rbp.tile([D, 512], F32, name="rbc", tag="rbc")
                    rinv = rbp.tile([D, 512], F32, name="rinv", tag="rinv")
                    nc.gpsimd.partition_broadcast(rbc[:], r_sb[:])
                    nc.vector.reciprocal_approx_fast(rinv[:], rbc[:])
                    nc.vector.tensor_mul(y_dst, y_ps[0:D, :], rinv[:])

        def proj_chunk(j, s, tok0):
            # partial out^T = W_p_local^T @ y_local for chunk j, stored into
            # ship s's RS buffer at token offset tok0. Rows of rs_in are the
            # full 1024 out channels = 4 ranks x 256, so the 4-core RS hands
            # each rank exactly its own channel slice, summed over the group.
            for oc in range(8):
                ps = psM.tile([128, 512], F32, name="psP", tag="psM")
                for i in range(2):
                    nc.tensor.matmul(
                        ps[:],
                        Wp[i][:, 128 * oc:128 * oc + 128],
                        yT[i][j][:],
                        start=(i == 0), stop=(i == 1))
                ob = osb.tile([128, 512], BF16, name="o_sb", tag="o_sb")
                nc.vector.tensor_copy(ob[:], ps[:])
                nc.sync.dma_start(
                    rs_in[s][128 * oc:128 * oc + 128, tok0:tok0 + 512],
                    ob[:])

        def rs_ship(s):
            nc.gpsimd.collective_compute(
                "ReduceScatter", mybir.AluOpType.add,
                replica_groups=RG4,
                ins=[rs_in[s].opt()], outs=[rs_out[s].opt()])

        # QKV chunk-groups feed directly into their attention chunks: the
        # dense QKV/proj matmuls interleave with the ACT-bound attention so
        # the PE never idles long enough for HAM to re-throttle.
        with tc.tile_pool(name="psS", bufs=2, space="PSUM") as psS, \
             tc.tile_pool(name="psY", bufs=2, space="PSUM") as psY, \
             tc.tile_pool(name="psM", bufs=2, space="PSUM") as psM:
            # qk(j+1) is emitted BEFORE proj(j): at a chunk boundary the PE
            # then never waits on proj's PSUM evacuations
            qk_group(0)
            v_group(0)
            attn_chunk(0)
            qk_group(1)
            proj_chunk(0, 0, 0)
            v_group(1)
            attn_chunk(1)
            qk_group(2)
            proj_chunk(1, 0, 512)
            v_group(2)
            attn_chunk(2)
            qk_group(3)
            proj_chunk(2, 0, 1024)
            rs_ship(0)
            v_group(3)
            attn_chunk(3)
            proj_chunk(3, 1, 0)
            # Ship 0's result bounces DRAM->SBUF->DRAM (loads on Sync, stores
            # on Scalar). Safe against head-of-line blocking because RS0
            # completes only after the last exp retires and after proj3's
            # stores are nearly done; the SBUF hops keep the engine charge at
            # ~500ns each (a DRAM->DRAM copy would occupy its engine 19us).
            w = rs_w[0]
            ob0 = osb.tile([128, 2 * w], BF16, name="rs_ob0", tag="rs_ob0")
            for i in range(2):
                nc.sync.dma_start(ob0[:, w * i:w * i + w],
                                  rs_out[0][128 * i:128 * i + 128, :])
                nc.scalar.dma_start(outs[0][128 * i:128 * i + 128, :],
                                    ob0[:, w * i:w * i + w])
            rs_ship(1)
            # tail bounce for ship 1: two pipelined halves, load on Sync,
            # store on Scalar (both provably idle by then)
            w = rs_w[1]
            ob = osb.tile([128, 2 * w], BF16, name="rs_ob", tag="rs_ob")
            for i in range(2):
                nc.sync.dma_start(ob[:, w * i:w * i + w],
                                  rs_out[1][128 * i:128 * i + 128, :])
                nc.scalar.dma_start(outs[1][128 * i:128 * i + 128, :],
                                    ob[:, w * i:w * i + w])

    nc.compile()
    return nc


def shard_inputs(x, W_attn, b_attn, W_proj, b_proj):
    scale = np.float32(D ** -0.5)
    bf16 = mybir.dt.np(BF16)
    in_maps = []
    for c in range(NCORES):
        b, g = divmod(c, TP)
        q = slice(CH * g, CH * (g + 1))
        k = slice(C + CH * g, C + CH * (g + 1))
        v = slice(2 * C + CH * g, 2 * C + CH * (g + 1))
        W_qk = np.concatenate([W_attn[:, q] * scale, W_attn[:, k]], axis=1)
        b_qk = np.concatenate([b_attn[q] * scale, b_attn[k]])
        in_maps.append({
            "x_t": np.ascontiguousarray(x[b].T, dtype=np.float32),
            "w_qk": np.ascontiguousarray(W_qk, dtype=np.float32),
            "b_qk": np.ascontiguousarray(b_qk, dtype=np.float32),
            "w_v": np.ascontiguousarray(W_attn[:, v], dtype=np.float32),
            "b_v": np.ascontiguousarray(b_attn[v], dtype=np.float32),
            "w_p": np.ascontiguousarray(
                W_proj[CH * g:CH * (g + 1), :]).astype(bf16),
        })
    return in_maps


def assemble(per_core_outs, b_proj):
    """per_core_outs: list of 8 dicts with o0/o1 out^T bf16 slices."""
    out = np.zeros((B, T, C), dtype=np.float32)
    for c in range(NCORES):
        b, g = divmod(c, TP)
        y_t = np.concatenate(
            [np.asarray(per_core_outs[c][f"o{s}"], dtype=np.float32)
             for s in range(2)], axis=1)          # [256, 2048]
        out[b, :, CH * g:CH * (g + 1)] = y_t.T
    out += np.asarray(b_proj, dtype=np.float32)[None, None, :]
    return out


_NC_CACHE = {}


def get_compiled():
    if "nc" not in _NC_CACHE:
        _NC_CACHE["nc"] = build_kernel()
    return _NC_CACHE["nc"]


def run_on_hw(in_maps, **kwargs):
    nc = get_compiled()
    return bass_utils.run_bass_kernel_spmd(
        nc, in_maps, core_ids=list(range(NCORES)), **kwargs)


def kernel(x, W_attn, b_attn, W_proj, b_proj):
    x = np.asarray(x, dtype=np.float32)
    W_attn = np.asarray(W_attn, dtype=np.float32)
    b_attn = np.asarray(b_attn, dtype=np.float32)
    W_proj = np.asarray(W_proj, dtype=np.float32)
    b_proj = np.asarray(b_proj, dtype=np.float32)

    in_maps = shard_inputs(x, W_attn, b_attn, W_proj, b_proj)
    res = run_on_hw(in_maps)
    return assemble(res.results, b_proj)

